# revision 31
# baseline (speedup 1.0000x reference)
"""Trainium2 Bass kernel for the thin-plate-spline RBF layer.

reference:  out[b,n,d] = sum_m phi(|x_bn - c_bm|) * w[b,m,d],
            phi(r) = r^2 * log(r + 1e-6)

Device algorithm (per core, N sharded 8 ways):
  dist2[m,n] = sum_k a_k[m] * b_k[n]   -- rank-15 bf16 split-precision
      expansion of |x-c|^2 (coordinates centered, split into bf16 hi/lo;
      bf16 products are exact under fp32 PSUM accumulation).
      One matmul per 512-col block (nt, h, b); the four batches map to
      the four 32-row PE strips (tile_position) and run concurrently.
  L[m,n] = ln(dist2 + 5e-5)            (ScalarE, fp16 out)
  The elementwise dist2*L multiply is eliminated algebraically:
    out[b,n,d] = sum_k b_k[n] * S[(k,b,d), n],
    S = sum_m (0.5 * a_k[m] * w[m,d]) * L[m,n]   (TensorE fp16,
        batch-stacked block-diagonal weights, 60 columns).

Pipeline: blocks are ground into [128,1536] PSUM tiles (3 banks), Ln'd
as one ScalarE instruction each (fewer instructions amortize the
~352-cycle ACT overhead), and consumed by per-n-tile S chains as soon
as each L block lands.  ScalarE is the bottleneck engine (~34 us of Ln
at 1 elem/cycle/lane); everything else hides behind it.  A junk-matmul
burst at t=0 warms the PE HAM clock gate while input DMAs land.
"""
import sys

sys.path.insert(0, "/opt/trn_rl_repo")

import numpy as np
import ml_dtypes

BF16 = np.dtype(ml_dtypes.bfloat16)
FP16 = np.float16

B, M, N, NCORES = 4, 256, 32768, 8
NS = N // NCORES          # 4096 dense points per core
NT = 512                  # n-tile (one PSUM bank of fp32)
NTILES = NS // NT         # 8
HALVES = M // 128         # 2
NBLK = B * HALVES         # 8 contraction blocks of 128 per n-tile
KD = 15                   # dist2 split-precision rank
J = 5 * B * 3             # 60 stacked S columns, j = k*12 + b*3 + d
JP = 64                   # padded stationary width
DELTA = 5e-5
TW = 3                    # 512-col blocks per d2/ACT tile
NWARM = 16                # junk warmup matmuls: 2 full HAM windows, so the
                          # PE clock gate opens on EVERY core (free-running
                          # window phase differs per core)

_compiled = None


def _build_nc():
    import concourse.bacc as bacc
    import concourse.mybir as mybir
    from concourse.tile import TileContext

    f32 = mybir.dt.float32
    bf = mybir.dt.bfloat16
    f16 = mybir.dt.float16
    nc = bacc.Bacc("TRN2")

    daug_d = nc.dram_tensor("daug", [128, NS], bf, kind="ExternalInput")
    cpa_d = nc.dram_tensor("cpa", [128, HALVES * 128], bf, kind="ExternalInput")
    wps_d = nc.dram_tensor("wps", [128, NBLK * JP], f16, kind="ExternalInput")
    rmat_d = nc.dram_tensor("rmat", [128, 12], f16, kind="ExternalInput")
    bcs_d = nc.dram_tensor("bcs", [128, NS], f32, kind="ExternalInput")
    out_d = nc.dram_tensor("outb", [12, NS], f32, kind="ExternalOutput")

    # block stream: one 512-col block per (n-tile, h, b).  Within an
    # n-tile PAIR g the slot order rotates b (4-way PE row-strip
    # concurrency for the dist2 matmuls) while alternating the n-tile
    # parity p (so consecutive S-chain matmuls target opposite PE
    # column groups and overlap).  PSUM accumulation commutes, so the
    # chain-matmul order within a chain is free.
    NG = NTILES // 2
    SLOTS = [(0, 0), (1, 1), (2, 0), (3, 1), (0, 1), (1, 0), (2, 1), (3, 0)]
    blocks = [(2 * g + p, h, b)
              for g in range(NG) for h in range(HALVES) for (b, p) in SLOTS]
    # uniform 2-block ACT tiles (two 2-bank PSUM pools ping-ponging);
    # the 8th PSUM bank is a dedicated junk-pad bank for HAM keep-warm.
    tiles = [(2 * t, 2) for t in range(len(blocks) // 2)]
    ntiles_act = len(tiles)

    with TileContext(nc) as tc:
        with (
            tc.tile_pool(name="singles", bufs=1) as singles,
            tc.tile_pool(name="lpool", bufs=6) as lpool,
            tc.tile_pool(name="zpool", bufs=3) as zpool,
            tc.tile_pool(name="d2poolA", bufs=1, space="PSUM") as d2poolA,
            tc.tile_pool(name="d2poolB", bufs=1, space="PSUM") as d2poolB,
            tc.tile_pool(name="spoolA", bufs=1, space="PSUM") as spoolA,
            tc.tile_pool(name="spoolB", bufs=1, space="PSUM") as spoolB,
            tc.tile_pool(name="opool", bufs=1, space="PSUM") as opool,
            tc.tile_pool(name="padpool", bufs=1, space="PSUM") as padpool,
        ):
            delta_t = singles.tile([128, 1], f32)
            nc.gpsimd.memset(delta_t, DELTA)
            scratch = singles.tile([128, NT], f16)
            nc.gpsimd.memset(scratch[:], 0.0)
            # dummy activation: hoists the ~2.6us ACT table load+drain to
            # t~7us (overlapping the input DMAs) instead of before ACT(0)
            dummy = singles.tile([128, 1], f32)
            nc.scalar.activation(
                out=dummy, in_=delta_t,
                func=mybir.ActivationFunctionType.Ln,
                bias=delta_t[:], scale=1.0,
            )

            # inputs, most-urgent first, spread over the two DGE queues
            daug_t = singles.tile([128, NS], bf)
            cpa_t = singles.tile([128, HALVES * 128], bf)
            wps_t = singles.tile([128, NBLK * JP], f16)
            rmat_t = singles.tile([128, 12], f16)
            bcs_t = singles.tile([128, NS], f32)
            out_sb = singles.tile([128, NS], f32)

            def dchunk(q, c):
                csl = slice(c * NT, (c + 1) * NT)
                q.dma_start(out=daug_t[:, csl], in_=daug_d[:, csl])

            nc.sync.dma_start(out=cpa_t[:], in_=cpa_d[:])
            dchunk(nc.sync, 0)
            dchunk(nc.gpsimd, 1)
            nc.gpsimd.dma_start(out=wps_t[:], in_=wps_d[:])
            dchunk(nc.sync, 2)
            nc.gpsimd.dma_start(out=rmat_t[:], in_=rmat_d[:])
            for c in range(3, NTILES):
                dchunk(nc.sync, c)
            for c in range(NTILES):
                csl = slice(c * NT, (c + 1) * NT)
                nc.gpsimd.dma_start(out=bcs_t[:, csl], in_=bcs_d[:, csl])

            # HAM warmup: junk matmul burst while input DMAs land.  All
            # junk lives in its own PSUM bank (no shared-bank deps).
            pad_t = padpool.tile([128, NT], f32, tag="pad", name="pad_t")

            def pad_mm(n, w=NT):
                for _ in range(n):
                    nc.tensor.matmul(
                        pad_t[:, :w], scratch[:, :128], scratch[:, :w],
                        start=True, stop=True,
                    )

            pad_mm(NWARM)

            # two-tile software pipeline: d2(t) is emitted (and thus sits in
            # the PE queue) ahead of chains(t-2), so a cold-PE chain backlog
            # can never starve ScalarE of its next d2 tile.
            def emit_d2(t):
                start, w = tiles[t]
                pool, tg = (d2poolA, "d2A") if t % 2 == 0 else (d2poolB, "d2B")
                d2 = pool.tile([128, w * NT], f32, tag=tg, name=f"d2_{t}")
                for j, (k, h, b) in enumerate(blocks[start:start + w]):
                    nsl = slice(k * NT, (k + 1) * NT)
                    nc.tensor.matmul(
                        d2[:, j * NT:(j + 1) * NT],
                        cpa_t[32 * b: 32 * b + KD, h * 128:(h + 1) * 128],
                        daug_t[32 * b: 32 * b + KD, nsl],
                        start=True,
                        stop=True,
                        tile_position=(32 * b, 0),
                    )
                return d2

            state = {0: None, 1: None}   # parity -> live s_c tile
            nmm = [0] * NTILES           # chain matmuls emitted per n-tile
            deferred = []                # (k, z_t) finalizations to flush

            def flush_deferred():
                # rmat + copy + out-DMA run one ACT tile late: by then the
                # z-mul they wait on is long done, so the rmat matmul never
                # head-of-line-blocks the PE queue in front of live chains.
                while deferred:
                    k, z_t = deferred.pop(0)
                    base = 64 * (k % 2)
                    nsl = slice(k * NT, (k + 1) * NT)
                    o2 = opool.tile([128, NT], f32, tag="o2", name=f"o2_{k}")
                    nc.tensor.matmul(
                        o2[base:base + 12, :],
                        rmat_t[base:base + J, :],
                        z_t[base:base + J, :],
                        start=True, stop=True,
                        tile_position=(base, base),
                    )
                    nc.vector.tensor_copy(out_sb[base:base + 12, nsl],
                                          o2[base:base + 12, :])
                    nc.sync.dma_start(out=out_d[:, nsl],
                                      in_=out_sb[base:base + 12, nsl])

            def emit_act_and_chains(t, d2):
                start, w = tiles[t]
                flush_deferred()
                lt = lpool.tile([128, 2 * NT], f16, tag="L", name=f"L{t}")
                nc.scalar.activation(
                    out=lt[:, :w * NT],
                    in_=d2[:],
                    func=mybir.ActivationFunctionType.Ln,
                    bias=delta_t[:],
                    scale=1.0,
                )
                for j, (k, h, b) in enumerate(blocks[start:start + w]):
                    l = 4 * h + b
                    p = k % 2                    # n-tile parity (col group)
                    if nmm[k] == 0:
                        if p == 0:
                            state[0] = spoolA.tile(
                                [JP, NT], f32, tag="SA", name=f"s_cA{k}")
                        else:
                            state[1] = spoolB.tile(
                                [128, NT], f32, tag="SB", name=f"s_cB{k}")
                    s_c = state[p]
                    out_ap = s_c[:] if p == 0 else s_c[64:128, :]
                    nc.tensor.matmul(
                        out_ap,
                        wps_t[:, l * JP:(l + 1) * JP],
                        lt[:, j * NT:(j + 1) * NT],
                        start=(nmm[k] == 0),
                        stop=(nmm[k] == NBLK - 1),
                        tile_position=(0, 64 * p),
                    )
                    nmm[k] += 1
                    if nmm[k] == NBLK:
                        nsl = slice(k * NT, (k + 1) * NT)
                        base = 64 * p
                        z_t = zpool.tile([128, NT], f16, tag="z",
                                         name=f"z{k}")
                        nc.vector.tensor_mul(
                            z_t[base:base + J, :],
                            s_c[base:base + J, :],
                            bcs_t[base:base + J, nsl])
                        deferred.append((k, z_t))
                # HAM keep-warm padding: full-array junk matmuls so the
                # clock gate stays at 2.4 GHz.  Reading lt pins them after
                # ACT(t) in the schedule (they have no other deps and the
                # list scheduler would otherwise float them to the start).
                if 2 <= t:
                    for _ in range(2):
                        nc.tensor.matmul(
                            pad_t[:, :448], scratch[:, :128], lt[:, :448],
                            start=True, stop=True,
                        )

            pending = []
            for t in range(ntiles_act):
                pending.append((t, emit_d2(t)))
                if len(pending) > 2:
                    emit_act_and_chains(*pending.pop(0))
            while pending:
                emit_act_and_chains(*pending.pop(0))
            flush_deferred()

    nc.compile()
    return nc


def _split3(v):
    """3-way bf16 split of float64 array."""
    hi = v.astype(BF16)
    r1 = v - hi.astype(np.float64)
    mid = r1.astype(BF16)
    r2 = r1 - mid.astype(np.float64)
    lo = r2.astype(BF16)
    return hi, mid, lo


def _host_prep(sparse_disp, original_cp, original_dense):
    """Build per-core input maps for the device kernel."""
    x = original_dense.astype(np.float64) - 0.5   # (B, N, 3) centered
    c = original_cp.astype(np.float64) - 0.5      # (B, M, 3)
    w = sparse_disp.astype(np.float32)            # (B, M, 3)

    # ---- control-point side (shared by all cores) ----
    p = c.astype(BF16)
    q = (c - p.astype(np.float64)).astype(BF16)
    t_hi, t_mid, t_lo = _split3((c * c).sum(-1))
    ones_m = np.ones((B, M), BF16)

    # per-batch KD rows: [p x3, p x3, q x3, t_hi, t_mid, t_lo, 1, 1, 1]
    cpa_full = np.empty((B, KD, M), BF16)
    for d in range(3):
        cpa_full[:, d, :] = p[:, :, d]
        cpa_full[:, 3 + d, :] = p[:, :, d]
        cpa_full[:, 6 + d, :] = q[:, :, d]
    cpa_full[:, 9, :] = t_hi
    cpa_full[:, 10, :] = t_mid
    cpa_full[:, 11, :] = t_lo
    cpa_full[:, 12, :] = ones_m
    cpa_full[:, 13, :] = ones_m
    cpa_full[:, 14, :] = ones_m

    # stacked stationary: rows 32b..32b+KD, cols h*128..
    cpa = np.zeros((128, HALVES * 128), BF16)
    for b in range(B):
        for h in range(HALVES):
            cpa[32 * b: 32 * b + KD, h * 128:(h + 1) * 128] = \
                cpa_full[b, :, h * 128:(h + 1) * 128]

    # S-chain stationaries, fp16, l = 4h + b, packed side by side
    wps = np.zeros((128, NBLK * JP), FP16)
    c32 = c.astype(np.float32)
    a5 = np.stack(
        [c32[:, :, 0], c32[:, :, 1], c32[:, :, 2],
         (c32 * c32).sum(-1), np.ones((B, M), np.float32)],
        axis=1,
    )  # (B, 5, M)
    for h in range(HALVES):
        for b in range(B):
            l = 4 * h + b
            msl = slice(h * 128, (h + 1) * 128)
            for k in range(5):
                for d in range(3):
                    j = k * 12 + b * 3 + d
                    wps[:, l * JP + j] = 0.5 * a5[b, k, msl] * w[b, msl, d]

    # k-reduction matrix, rows duplicated at partition bases 0 and 64
    # (even / odd n-tile chains live in different partition halves)
    rmat = np.zeros((128, 12), FP16)
    for j in range(J):
        rmat[j, j % 12] = 1.0
        rmat[64 + j, j % 12] = 1.0

    # ---- dense-point side (per core) ----
    u_all = x.astype(BF16)
    v_all = (x - u_all.astype(np.float64)).astype(BF16)
    s_all = (x * x).sum(-1)

    in_maps = []
    for core in range(NCORES):
        csl = slice(core * NS, (core + 1) * NS)
        u = u_all[:, csl, :].astype(np.float32)
        v = v_all[:, csl, :].astype(np.float32)
        s_hi, s_mid, s_lo = _split3(s_all[:, csl])
        ones_n = np.ones((B, NS), BF16)

        daug_b = np.empty((B, KD, NS), BF16)
        for d in range(3):
            daug_b[:, d, :] = (-2.0 * u[:, :, d]).astype(BF16)
            daug_b[:, 3 + d, :] = (-2.0 * v[:, :, d]).astype(BF16)
            daug_b[:, 6 + d, :] = (-2.0 * u[:, :, d]).astype(BF16)
        daug_b[:, 9, :] = ones_n
        daug_b[:, 10, :] = ones_n
        daug_b[:, 11, :] = ones_n
        daug_b[:, 12, :] = s_hi
        daug_b[:, 13, :] = s_mid
        daug_b[:, 14, :] = s_lo

        daug = np.zeros((128, NS), BF16)
        for b in range(B):
            daug[32 * b: 32 * b + KD] = daug_b[b]

        xs = x[:, csl, :].astype(np.float32)
        baug5 = np.stack(
            [-2.0 * xs[:, :, 0], -2.0 * xs[:, :, 1], -2.0 * xs[:, :, 2],
             np.ones((B, NS), np.float32), (xs * xs).sum(-1)],
            axis=1,
        )  # (B, 5, NS)
        bc = np.zeros((128, NS), np.float32)
        for k in range(5):
            for b in range(B):
                for d in range(3):
                    bc[k * 12 + b * 3 + d] = baug5[b, k]
        bc[64:64 + J] = bc[:J]

        in_maps.append(
            {
                "daug": daug,
                "bcs": bc,
                "cpa": cpa,
                "wps": wps,
                "rmat": rmat,
            }
        )
    return in_maps


def _assemble(results):
    out = np.empty((B, N, 3), np.float32)
    for core, r in enumerate(results):
        o = r["outb"]  # (12, NS) rows b*3+d
        out[:, core * NS:(core + 1) * NS, :] = (
            o.reshape(B, 3, NS).transpose(0, 2, 1)
        )
    return out


def kernel(sparse_disp, original_cp, original_dense):
    global _compiled
    from concourse.bass_utils import run_bass_kernel_spmd

    if _compiled is None:
        _compiled = _build_nc()
    in_maps = _host_prep(sparse_disp, original_cp, original_dense)
    res = run_bass_kernel_spmd(_compiled, in_maps, core_ids=list(range(NCORES)))
    return _assemble(res.results)


# revision 33
# speedup vs baseline: 1.1825x; 1.1825x over previous
"""Trainium2 Bass kernel for the thin-plate-spline RBF layer.

reference:  out[b,n,d] = sum_m phi(|x_bn - c_bm|) * w[b,m,d],
            phi(r) = r^2 * log(r + 1e-6)

Device algorithm (per core, N sharded 8 ways):
  dist2[m,n] = sum_k a_k[m] * b_k[n]   -- rank-15 bf16 split-precision
      expansion of |x-c|^2 (coordinates centered, split into bf16 hi/lo;
      bf16 products are exact under fp32 PSUM accumulation).
      One matmul per 512-col block (nt, h, b); the four batches map to
      the four 32-row PE strips (tile_position) and run concurrently.
  L[m,n] = ln(dist2 + 5e-5)            (ScalarE, fp16 out)
  The elementwise dist2*L multiply is eliminated algebraically:
    out[b,n,d] = sum_k b_k[n] * S[(k,b,d), n],
    S = sum_m (0.5 * a_k[m] * w[m,d]) * L[m,n]   (TensorE fp16,
        batch-stacked block-diagonal weights, 60 columns).

Pipeline: blocks are ground into [128,1536] PSUM tiles (3 banks), Ln'd
as one ScalarE instruction each (fewer instructions amortize the
~352-cycle ACT overhead), and consumed by per-n-tile S chains as soon
as each L block lands.  ScalarE is the bottleneck engine (~34 us of Ln
at 1 elem/cycle/lane); everything else hides behind it.  A junk-matmul
burst at t=0 warms the PE HAM clock gate while input DMAs land.
"""
import sys

sys.path.insert(0, "/opt/trn_rl_repo")

import numpy as np
import ml_dtypes

BF16 = np.dtype(ml_dtypes.bfloat16)
FP16 = np.float16

B, M, N, NCORES = 4, 256, 32768, 8
NS = N // NCORES          # 4096 dense points per core
NT = 512                  # n-tile (one PSUM bank of fp32)
NTILES = NS // NT         # 8
HALVES = M // 128         # 2
NBLK = B * HALVES         # 8 contraction blocks of 128 per n-tile
KD = 15                   # dist2 split-precision rank
J = 5 * B * 3             # 60 stacked S columns, j = k*12 + b*3 + d
JP = 64                   # padded stationary width
DELTA = 5e-5
TW = 3                    # 512-col blocks per d2/ACT tile
NWARM = 16                # junk warmup matmuls: 2 full HAM windows, so the
                          # PE clock gate opens on EVERY core (free-running
                          # window phase differs per core)

_compiled = None


def _build_nc():
    import concourse.bacc as bacc
    import concourse.mybir as mybir
    from concourse.tile import TileContext

    f32 = mybir.dt.float32
    bf = mybir.dt.bfloat16
    f16 = mybir.dt.float16
    nc = bacc.Bacc("TRN2")

    daug_d = nc.dram_tensor("daug", [128, NS], bf, kind="ExternalInput")
    cpa_d = nc.dram_tensor("cpa", [128, HALVES * 128], bf, kind="ExternalInput")
    wps_d = nc.dram_tensor("wps", [128, NBLK * JP], f16, kind="ExternalInput")
    rmat_d = nc.dram_tensor("rmat", [128, 12], f16, kind="ExternalInput")
    bcs_d = nc.dram_tensor("bcs", [128, NS], f32, kind="ExternalInput")
    out_d = nc.dram_tensor("outb", [12, NS], f32, kind="ExternalOutput")

    # block stream: one 512-col block per (n-tile, h, b).  Within an
    # n-tile PAIR g the slot order rotates b (4-way PE row-strip
    # concurrency for the dist2 matmuls) while alternating the n-tile
    # parity p (so consecutive S-chain matmuls target opposite PE
    # column groups and overlap).  PSUM accumulation commutes, so the
    # chain-matmul order within a chain is free.
    NG = NTILES // 2
    SLOTS = [(0, 0), (1, 1), (2, 0), (3, 1), (0, 1), (1, 0), (2, 1), (3, 0)]
    blocks = [(2 * g + p, h, b)
              for g in range(NG) for h in range(HALVES) for (b, p) in SLOTS]
    # uniform 2-block ACT tiles (two 2-bank PSUM pools ping-ponging);
    # the 8th PSUM bank is a dedicated junk-pad bank for HAM keep-warm.
    tiles = [(2 * t, 2) for t in range(len(blocks) // 2)]
    ntiles_act = len(tiles)

    with TileContext(nc) as tc:
        with (
            tc.tile_pool(name="singles", bufs=1) as singles,
            tc.tile_pool(name="lpool", bufs=6) as lpool,
            tc.tile_pool(name="zpool", bufs=3) as zpool,
            tc.tile_pool(name="d2poolA", bufs=1, space="PSUM") as d2poolA,
            tc.tile_pool(name="d2poolB", bufs=1, space="PSUM") as d2poolB,
            tc.tile_pool(name="spoolA", bufs=1, space="PSUM") as spoolA,
            tc.tile_pool(name="spoolB", bufs=1, space="PSUM") as spoolB,
            tc.tile_pool(name="opool", bufs=1, space="PSUM") as opool,
            tc.tile_pool(name="padpool", bufs=1, space="PSUM") as padpool,
        ):
            delta_t = singles.tile([128, 1], f32)
            nc.gpsimd.memset(delta_t, DELTA)
            scratch = singles.tile([128, NT], f16)
            nc.gpsimd.memset(scratch[:], 0.0)
            # dummy activation: hoists the ~2.6us ACT table load+drain to
            # t~7us (overlapping the input DMAs) instead of before ACT(0)
            dummy = singles.tile([128, 1], f32)
            nc.scalar.activation(
                out=dummy, in_=delta_t,
                func=mybir.ActivationFunctionType.Ln,
                bias=delta_t[:], scale=1.0,
            )

            # inputs, most-urgent first, spread over the two DGE queues
            daug_t = singles.tile([128, NS], bf)
            cpa_t = singles.tile([128, HALVES * 128], bf)
            wps_t = singles.tile([128, NBLK * JP], f16)
            rmat_t = singles.tile([128, 12], f16)
            bcs_t = singles.tile([128, NS], f32)
            out_sb = singles.tile([128, NS], f32)

            def dchunk(q, c):
                csl = slice(c * NT, (c + 1) * NT)
                q.dma_start(out=daug_t[:, csl], in_=daug_d[:, csl])

            nc.sync.dma_start(out=cpa_t[:], in_=cpa_d[:])
            dchunk(nc.sync, 0)
            dchunk(nc.gpsimd, 1)
            nc.gpsimd.dma_start(out=wps_t[:], in_=wps_d[:])
            dchunk(nc.sync, 2)
            nc.gpsimd.dma_start(out=rmat_t[:], in_=rmat_d[:])
            for c in range(3, NTILES):
                dchunk(nc.sync, c)
            for c in range(NTILES):
                csl = slice(c * NT, (c + 1) * NT)
                nc.gpsimd.dma_start(out=bcs_t[:, csl], in_=bcs_d[:, csl])

            # HAM warmup: junk matmul burst while input DMAs land.  All
            # junk lives in its own PSUM bank (no shared-bank deps).
            pad_t = padpool.tile([128, NT], f32, tag="pad", name="pad_t")

            def pad_mm(n, w=NT):
                for _ in range(n):
                    nc.tensor.matmul(
                        pad_t[:, :w], scratch[:, :128], scratch[:, :w],
                        start=True, stop=True,
                    )

            pad_mm(NWARM)

            # two-tile software pipeline: d2(t) is emitted (and thus sits in
            # the PE queue) ahead of chains(t-2), so a cold-PE chain backlog
            # can never starve ScalarE of its next d2 tile.
            def emit_d2(t):
                start, w = tiles[t]
                pool, tg = (d2poolA, "d2A") if t % 2 == 0 else (d2poolB, "d2B")
                d2 = pool.tile([128, w * NT], f32, tag=tg, name=f"d2_{t}")
                for j, (k, h, b) in enumerate(blocks[start:start + w]):
                    nsl = slice(k * NT, (k + 1) * NT)
                    nc.tensor.matmul(
                        d2[:, j * NT:(j + 1) * NT],
                        cpa_t[32 * b: 32 * b + KD, h * 128:(h + 1) * 128],
                        daug_t[32 * b: 32 * b + KD, nsl],
                        start=True,
                        stop=True,
                        tile_position=(32 * b, 0),
                    )
                return d2

            state = {0: None, 1: None}   # parity -> live s_c tile
            nmm = [0] * NTILES           # chain matmuls emitted per n-tile
            deferred = []                # (k, z_t) finalizations to flush
            lt_hist = []                 # recent L tiles (pad anchoring)

            def flush_deferred():
                # rmat + copy + out-DMA run one ACT tile late: by then the
                # z-mul they wait on is long done, so the rmat matmul never
                # head-of-line-blocks the PE queue in front of live chains.
                while deferred:
                    k, z_t = deferred.pop(0)
                    base = 64 * (k % 2)
                    nsl = slice(k * NT, (k + 1) * NT)
                    o2 = opool.tile([128, NT], f32, tag="o2", name=f"o2_{k}")
                    nc.tensor.matmul(
                        o2[base:base + 12, :],
                        rmat_t[base:base + J, :],
                        z_t[base:base + J, :],
                        start=True, stop=True,
                        tile_position=(base, base),
                    )
                    nc.vector.tensor_copy(out_sb[base:base + 12, nsl],
                                          o2[base:base + 12, :])
                    nc.sync.dma_start(out=out_d[:, nsl],
                                      in_=out_sb[base:base + 12, nsl])

            def emit_act_and_chains(t, d2):
                start, w = tiles[t]
                flush_deferred()
                lt = lpool.tile([128, 2 * NT], f16, tag="L", name=f"L{t}")
                nc.scalar.activation(
                    out=lt[:, :w * NT],
                    in_=d2[:],
                    func=mybir.ActivationFunctionType.Ln,
                    bias=delta_t[:],
                    scale=1.0,
                )
                for j, (k, h, b) in enumerate(blocks[start:start + w]):
                    l = 4 * h + b
                    p = k % 2                    # n-tile parity (col group)
                    if nmm[k] == 0:
                        if p == 0:
                            state[0] = spoolA.tile(
                                [JP, NT], f32, tag="SA", name=f"s_cA{k}")
                        else:
                            state[1] = spoolB.tile(
                                [128, NT], f32, tag="SB", name=f"s_cB{k}")
                    s_c = state[p]
                    out_ap = s_c[:] if p == 0 else s_c[64:128, :]
                    nc.tensor.matmul(
                        out_ap,
                        wps_t[:, l * JP:(l + 1) * JP],
                        lt[:, j * NT:(j + 1) * NT],
                        start=(nmm[k] == 0),
                        stop=(nmm[k] == NBLK - 1),
                        tile_position=(0, 64 * p),
                    )
                    nmm[k] += 1
                    if nmm[k] == NBLK:
                        nsl = slice(k * NT, (k + 1) * NT)
                        base = 64 * p
                        z_t = zpool.tile([128, NT], f16, tag="z",
                                         name=f"z{k}")
                        nc.vector.tensor_mul(
                            z_t[base:base + J, :],
                            s_c[base:base + J, :],
                            bcs_t[base:base + J, nsl])
                        deferred.append((k, z_t))
                # HAM keep-warm padding: full-array junk matmuls so the
                # clock gate stays at 2.4 GHz.  Each pad reads the L tile
                # from TWO tiles ago: that dependency is long satisfied (so
                # the pad never head-of-line-blocks the PE queue) but stops
                # the list scheduler from floating all pads to the start.
                lt_hist.append(lt)
                if len(lt_hist) > 2:
                    old = lt_hist.pop(0)
                    for _ in range(2):
                        nc.tensor.matmul(
                            pad_t[:, :256], scratch[:, :128], old[:, :256],
                            start=True, stop=True,
                        )

            pending = []
            for t in range(ntiles_act):
                pending.append((t, emit_d2(t)))
                if len(pending) > 2:
                    emit_act_and_chains(*pending.pop(0))
            while pending:
                emit_act_and_chains(*pending.pop(0))
            flush_deferred()

    nc.compile()
    return nc


def _split3(v):
    """3-way bf16 split of float64 array."""
    hi = v.astype(BF16)
    r1 = v - hi.astype(np.float64)
    mid = r1.astype(BF16)
    r2 = r1 - mid.astype(np.float64)
    lo = r2.astype(BF16)
    return hi, mid, lo


def _host_prep(sparse_disp, original_cp, original_dense):
    """Build per-core input maps for the device kernel."""
    x = original_dense.astype(np.float64) - 0.5   # (B, N, 3) centered
    c = original_cp.astype(np.float64) - 0.5      # (B, M, 3)
    w = sparse_disp.astype(np.float32)            # (B, M, 3)

    # ---- control-point side (shared by all cores) ----
    p = c.astype(BF16)
    q = (c - p.astype(np.float64)).astype(BF16)
    t_hi, t_mid, t_lo = _split3((c * c).sum(-1))
    ones_m = np.ones((B, M), BF16)

    # per-batch KD rows: [p x3, p x3, q x3, t_hi, t_mid, t_lo, 1, 1, 1]
    cpa_full = np.empty((B, KD, M), BF16)
    for d in range(3):
        cpa_full[:, d, :] = p[:, :, d]
        cpa_full[:, 3 + d, :] = p[:, :, d]
        cpa_full[:, 6 + d, :] = q[:, :, d]
    cpa_full[:, 9, :] = t_hi
    cpa_full[:, 10, :] = t_mid
    cpa_full[:, 11, :] = t_lo
    cpa_full[:, 12, :] = ones_m
    cpa_full[:, 13, :] = ones_m
    cpa_full[:, 14, :] = ones_m

    # stacked stationary: rows 32b..32b+KD, cols h*128..
    cpa = np.zeros((128, HALVES * 128), BF16)
    for b in range(B):
        for h in range(HALVES):
            cpa[32 * b: 32 * b + KD, h * 128:(h + 1) * 128] = \
                cpa_full[b, :, h * 128:(h + 1) * 128]

    # S-chain stationaries, fp16, l = 4h + b, packed side by side
    wps = np.zeros((128, NBLK * JP), FP16)
    c32 = c.astype(np.float32)
    a5 = np.stack(
        [c32[:, :, 0], c32[:, :, 1], c32[:, :, 2],
         (c32 * c32).sum(-1), np.ones((B, M), np.float32)],
        axis=1,
    )  # (B, 5, M)
    for h in range(HALVES):
        for b in range(B):
            l = 4 * h + b
            msl = slice(h * 128, (h + 1) * 128)
            for k in range(5):
                for d in range(3):
                    j = k * 12 + b * 3 + d
                    wps[:, l * JP + j] = 0.5 * a5[b, k, msl] * w[b, msl, d]

    # k-reduction matrix, rows duplicated at partition bases 0 and 64
    # (even / odd n-tile chains live in different partition halves)
    rmat = np.zeros((128, 12), FP16)
    for j in range(J):
        rmat[j, j % 12] = 1.0
        rmat[64 + j, j % 12] = 1.0

    # ---- dense-point side (per core) ----
    u_all = x.astype(BF16)
    v_all = (x - u_all.astype(np.float64)).astype(BF16)
    s_all = (x * x).sum(-1)

    in_maps = []
    for core in range(NCORES):
        csl = slice(core * NS, (core + 1) * NS)
        u = u_all[:, csl, :].astype(np.float32)
        v = v_all[:, csl, :].astype(np.float32)
        s_hi, s_mid, s_lo = _split3(s_all[:, csl])
        ones_n = np.ones((B, NS), BF16)

        daug_b = np.empty((B, KD, NS), BF16)
        for d in range(3):
            daug_b[:, d, :] = (-2.0 * u[:, :, d]).astype(BF16)
            daug_b[:, 3 + d, :] = (-2.0 * v[:, :, d]).astype(BF16)
            daug_b[:, 6 + d, :] = (-2.0 * u[:, :, d]).astype(BF16)
        daug_b[:, 9, :] = ones_n
        daug_b[:, 10, :] = ones_n
        daug_b[:, 11, :] = ones_n
        daug_b[:, 12, :] = s_hi
        daug_b[:, 13, :] = s_mid
        daug_b[:, 14, :] = s_lo

        daug = np.zeros((128, NS), BF16)
        for b in range(B):
            daug[32 * b: 32 * b + KD] = daug_b[b]

        xs = x[:, csl, :].astype(np.float32)
        baug5 = np.stack(
            [-2.0 * xs[:, :, 0], -2.0 * xs[:, :, 1], -2.0 * xs[:, :, 2],
             np.ones((B, NS), np.float32), (xs * xs).sum(-1)],
            axis=1,
        )  # (B, 5, NS)
        bc = np.zeros((128, NS), np.float32)
        for k in range(5):
            for b in range(B):
                for d in range(3):
                    bc[k * 12 + b * 3 + d] = baug5[b, k]
        bc[64:64 + J] = bc[:J]

        in_maps.append(
            {
                "daug": daug,
                "bcs": bc,
                "cpa": cpa,
                "wps": wps,
                "rmat": rmat,
            }
        )
    return in_maps


def _assemble(results):
    out = np.empty((B, N, 3), np.float32)
    for core, r in enumerate(results):
        o = r["outb"]  # (12, NS) rows b*3+d
        out[:, core * NS:(core + 1) * NS, :] = (
            o.reshape(B, 3, NS).transpose(0, 2, 1)
        )
    return out


def kernel(sparse_disp, original_cp, original_dense):
    global _compiled
    from concourse.bass_utils import run_bass_kernel_spmd

    if _compiled is None:
        _compiled = _build_nc()
    in_maps = _host_prep(sparse_disp, original_cp, original_dense)
    res = run_bass_kernel_spmd(_compiled, in_maps, core_ids=list(range(NCORES)))
    return _assemble(res.results)
